# revision 1
# baseline (speedup 1.0000x reference)
"""Trainium2 Bass kernel for nn_BayesianFlowNetworkDiscretised.

Computes, for each (b, d) position:
    MLP: h = gelu_tanh(W1[0,:]*mu + t*W1[1,:] + b1); (mu_eps, ln_sig) = h@W2 + b2
    mu_x = mu/gamma - var_scale*mu_eps
    sigma = max(var_scale*exp(ln_sig), 0.02)   [clip never binds for this data]
    out_k = Phi((e_k - mu_x)/sigma) - Phi((e_{k-1} - mu_x)/sigma),  e_i = i/8 - 1

Sharding: D split across 8 cores (data-parallel, no comm).
Per-core layout: partition p = b*4 + q holds mu[b, q*1536 : (q+1)*1536];
all per-b constants become per-partition [128,1] scale/bias vectors.

dtypes: fp16 for h / MLP accumulators / inv / erf outputs (error-analysed
safe: beta*inv <= ~1 bounds amplification); fp32 for mu, mu_x, final out.
"""

import sys

sys.path.insert(0, "/opt/trn_rl_repo")

import numpy as np

import concourse.bass as bass
import concourse.bacc as bacc
from concourse import mybir
from concourse.tile import TileContext
from concourse.bass_utils import run_bass_kernel_spmd

F32 = mybir.dt.float32
F16 = mybir.dt.float16
AF = mybir.ActivationFunctionType
OP = mybir.AluOpType

K = 16
SIGMA_ONE = 0.02
T_MIN = 1e-6
B, D, H = 32, 49152, 16
NCORES = 8
DS = D // NCORES          # 6144 columns per core
Q = 4                     # partitions per batch row
F = DS // Q               # 1536 free elements per partition
NCHUNK = 2                # output staging chunks
FC = F // NCHUNK          # 512
LN_SQRT2 = 0.34657359027997264


def _build(W1, b1, W2, b2):
    """Build the Bass module. Weights are baked in as immediates.

    The shard is processed in two column-halves forming a 2-stage software
    pipeline: half-2's gelu phase (ACT-heavy) overlaps half-1's
    args/erf/diffs phase (DVE-heavy).
    """
    nc = bacc.Bacc(None, target_bir_lowering=False)
    mu_p = nc.declare_dram_parameter("mu", [B, DS], F32, isOutput=False)
    cn_p = nc.declare_dram_parameter("cn", [128, H + 8], F32, isOutput=False)
    out_p = nc.declare_dram_parameter("out", [128, K, F], F32, isOutput=True)

    mu_v = mu_p.rearrange("b (q f) -> (b q) f", q=Q)
    HF = F // 2

    with TileContext(nc) as tc:
        with (
            tc.tile_pool(name="const", bufs=1) as constp,
            tc.tile_pool(name="main", bufs=1) as mainp,
            tc.tile_pool(name="tp", bufs=2) as tpool,
            tc.tile_pool(name="ph", bufs=2) as php,
            tc.tile_pool(name="hp", bufs=12) as hp,
            tc.tile_pool(name="fp", bufs=18) as fpool,
            tc.tile_pool(name="op", bufs=6) as opool,
        ):
            cn = constp.tile([128, H + 8], F32)
            nc.sync.dma_start(out=cn[:, :], in_=cn_p[:, :])
            cb = cn[:, 0:H]
            pb = cn[:, H : H + 8]
            mu = mainp.tile([128, F], F32)
            nc.sync.dma_start(out=mu[:, :], in_=mu_v)

            # ACT instructions support a single sync-wait slot; make the ACT
            # engine observe the const-DMA semaphore via a tiny copy so the
            # first gelu only needs to wait on the mu DMA.
            warm = constp.tile([128, 1], F32)
            nc.scalar.copy(out=warm[:, :], in_=cn[:, 0:1])

            alpha = pb[:, 0:1]      # 1/gamma            (0 if cond)
            negbeta = pb[:, 1:2]    # -var_scale         (0 if cond)
            lnA = pb[:, 2:3]        # ln(var_scale)      (-1e4 if cond)
            lnm = pb[:, 3:4]        # ln(0.02)           (0 if cond)

            for hf in range(2):
                sl = slice(hf * HF, (hf + 1) * HF)
                muh = mu[:, sl]

                # ---- phase A: gelu on ACT; W2-scaled copies + tree adds on DVE
                T_e = tpool.tile([128, H, HF], F16)
                T_l = tpool.tile([128, H, HF], F16)
                hsave = {}
                for j in range(H):
                    h = hp.tile([128, HF], F16)
                    nc.scalar.activation(
                        out=h, in_=muh, func=AF.Gelu_apprx_tanh,
                        bias=cb[:, j : j + 1], scale=float(W1[0, j]),
                    )
                    if j == 0:
                        nc.vector.tensor_scalar(
                            out=T_e[:, j, :], in0=h, scalar1=float(W2[j, 0]),
                            scalar2=float(b2[0]), op0=OP.mult, op1=OP.add)
                        nc.vector.tensor_scalar(
                            out=T_l[:, j, :], in0=h, scalar1=float(W2[j, 1]),
                            scalar2=float(b2[1]), op0=OP.mult, op1=OP.add)
                    else:
                        if j <= 5:
                            nc.vector.tensor_scalar_mul(
                                out=T_e[:, j, :], in0=h, scalar1=float(W2[j, 0]))
                        else:
                            hsave[j] = h
                        nc.vector.tensor_scalar_mul(
                            out=T_l[:, j, :], in0=h, scalar1=float(W2[j, 1]))

                # e-col scales for j>5 on ACT, after the gelu stream so they
                # don't delay it; DVE does trees meanwhile.
                for j in sorted(hsave):
                    nc.scalar.activation(
                        out=T_e[:, j, :], in_=hsave[j], func=AF.Copy,
                        scale=float(W2[j, 0]))

                # l-column first: it gates exp/inv (the critical path).
                for w in (8, 4, 2, 1):
                    nc.vector.tensor_tensor(
                        out=T_l[:, 0:w, :], in0=T_l[:, 0:w, :],
                        in1=T_l[:, w : 2 * w, :], op=OP.add)
                v = php.tile([128, HF], F16)
                nc.vector.tensor_scalar(
                    out=v, in0=T_l[:, 0, :], scalar1=lnA, scalar2=lnm,
                    op0=OP.add, op1=OP.max)
                inv = php.tile([128, HF], F16)
                nc.scalar.activation(
                    out=inv, in_=v, func=AF.Exp, scale=-1.0, bias=pb[:, 5:6])

                for w in (8, 4, 2, 1):
                    nc.vector.tensor_tensor(
                        out=T_e[:, 0:w, :], in0=T_e[:, 0:w, :],
                        in1=T_e[:, w : 2 * w, :], op=OP.add)

                # ---- mu_x = alpha*mu - beta*acc_e (acc_e already includes b2)
                mx = php.tile([128, HF], F32)
                nc.vector.tensor_scalar_mul(out=mx, in0=muh, scalar1=alpha)
                nc.vector.scalar_tensor_tensor(
                    out=mx, in0=T_e[:, 0, :], scalar=negbeta, in1=mx,
                    op0=OP.mult, op1=OP.add)

                # ---- args a_i = (e_i - mu_x)*inv via 3 anchors + stepping
                st = php.tile([128, HF], F16)
                nc.vector.tensor_scalar_mul(out=st, in0=inv, scalar1=0.125)
                ats = {}
                for i in (3, 8, 13):
                    a = fpool.tile([128, HF], F16)
                    nc.vector.tensor_scalar(
                        out=a, in0=mx, scalar1=-1.0,
                        scalar2=float(i / 8.0 - 1.0), op0=OP.mult, op1=OP.add)
                    nc.vector.tensor_mul(out=a, in0=a, in1=inv)
                    ats[i] = a
                for src_i, dst_i in ((3, 2), (2, 1), (3, 4), (4, 5),
                                     (8, 7), (7, 6), (8, 9), (9, 10),
                                     (13, 12), (12, 11), (13, 14), (14, 15)):
                    a = fpool.tile([128, HF], F16)
                    nc.vector.tensor_tensor(
                        out=a, in0=ats[src_i], in1=st,
                        op=OP.add if dst_i > src_i else OP.subtract)
                    ats[dst_i] = a
                # f_i = 0.5*erf(a_i), in place
                fts = []
                for i in range(1, 16):
                    fi = ats[i]
                    nc.scalar.activation(out=fi, in_=fi, func=AF.Erf)
                    nc.vector.tensor_scalar_mul(out=fi, in0=fi, scalar1=0.5)
                    fts.append(fi)

                # ---- diffs: contiguous per-k tiles, cast-DMA each k-plane
                for k2 in range(K):
                    o = opool.tile([128, HF], F16)
                    if k2 == 0:
                        nc.vector.tensor_scalar_add(
                            out=o, in0=fts[0], scalar1=0.5)
                    elif k2 == 15:
                        nc.vector.tensor_scalar(
                            out=o, in0=fts[14], scalar1=-1.0, scalar2=0.5,
                            op0=OP.mult, op1=OP.add)
                    else:
                        nc.vector.tensor_tensor(
                            out=o, in0=fts[k2], in1=fts[k2 - 1],
                            op=OP.subtract)
                    nc.gpsimd.dma_start(out=out_p[:, k2, sl], in_=o[:, :])

    return nc


def _host_consts(t, W1, b1, W2, b2):
    t = np.asarray(t, np.float64).reshape(B)
    cond = t < T_MIN
    gamma = 1.0 - SIGMA_ONE ** (2.0 * t)
    alpha = np.where(cond, 0.0, 1.0 / gamma)
    beta = np.sqrt(np.maximum(1.0 - gamma, 0.0) / gamma)
    negbeta = np.where(cond, 0.0, -beta)
    lnA = np.where(cond, -1e4, np.log(np.maximum(beta, 1e-300)))
    lnm = np.where(cond, 0.0, np.log(SIGMA_ONE))
    nb20 = np.where(cond, 0.0, -beta * float(b2[0]))

    pb = np.zeros((128, 8), np.float32)
    for b in range(B):
        for q in range(Q):
            p = b * Q + q
            pb[p, 0] = alpha[b]
            pb[p, 1] = negbeta[b]
            pb[p, 2] = lnA[b]
            pb[p, 3] = lnm[b]
            pb[p, 4] = nb20[b]
            pb[p, 5] = -LN_SQRT2

    cb = np.zeros((128, H), np.float32)
    cvals = t[:, None] * np.asarray(W1, np.float64)[1, :][None, :] + np.asarray(
        b1, np.float64)[None, :]                        # [B, H]
    for b in range(B):
        cb[b * Q : (b + 1) * Q, :] = cvals[b]
    return cb, pb


def _run(inputs, trace=False):
    mu = np.ascontiguousarray(np.asarray(inputs["mu"], np.float32))
    t = np.asarray(inputs["t"], np.float32)
    W1 = np.asarray(inputs["W1"], np.float32)
    b1 = np.asarray(inputs["b1"], np.float32)
    W2 = np.asarray(inputs["W2"], np.float32)
    b2 = np.asarray(inputs["b2"], np.float32)

    nc = _build(W1, b1, W2, b2)
    nc.finalize()
    cb, pb = _host_consts(t, W1, b1, W2, b2)

    cn = np.ascontiguousarray(np.concatenate([cb, pb], axis=1))
    in_maps = []
    for c in range(NCORES):
        shard = np.ascontiguousarray(mu[:, c * DS : (c + 1) * DS])
        in_maps.append({"mu": shard, "cn": cn})

    res = run_bass_kernel_spmd(nc, in_maps, list(range(NCORES)), trace=trace)
    shards = []
    for c in range(NCORES):
        s = np.asarray(res.results[c]["out"])          # [128, K, F]
        shards.append(s.reshape(B, Q, K, F).transpose(0, 1, 3, 2).reshape(B, DS, K))
    out = np.ascontiguousarray(np.concatenate(shards, axis=1))
    return out, res


def kernel(**inputs) -> np.ndarray:
    out, _ = _run(inputs, trace=False)
    return out


if __name__ == "__main__":
    rng = np.random.default_rng(0)
    demo = {
        "mu": rng.standard_normal((B, D), dtype=np.float32),
        "t": rng.random((B, 1), dtype=np.float32),
        "W1": rng.standard_normal((2, H), dtype=np.float32) * 0.5,
        "b1": rng.standard_normal((H,), dtype=np.float32) * 0.1,
        "W2": rng.standard_normal((H, 2), dtype=np.float32) * 0.1,
        "b2": rng.standard_normal((2,), dtype=np.float32) * 0.1,
    }
    out = kernel(**demo)
    print("kernel output", out.shape, out.dtype, out[0, 0])



# revision 3
# speedup vs baseline: 1.1735x; 1.1735x over previous
"""Trainium2 Bass kernel for nn_BayesianFlowNetworkDiscretised (v3).

Per (b, d) position:
    MLP: h_j = gelu_tanh(W1[0,j]*mu + t*W1[1,j] + b1[j]);  (mu_eps, ln_sig) = h@W2 + b2
    mu_x = mu/gamma - var_scale*mu_eps
    sigma = max(var_scale*exp(ln_sig), 0.02)
    out_k = Phi((e_{k+1}-mu_x)/sigma) - Phi((e_k-mu_x)/sigma),  e_i = i/8 - 1

Structure:
  - mu is partition-replicated 2x by DMA straight from DRAM (stride-2
    partition APs) into two expanded blocks; the per-unit W1 scale and the
    per-(batch,unit) bias ride free in the gelu ACTIVATE (per-partition
    scale/bias APs), so layer 1 costs nothing beyond the intrinsic gelus.
  - Layer 2 runs on the TensorEngine: per (block i, unit-pair u), bf16
    matmuls with W2-scatter stationaries accumulate eps/sig into PSUM in
    the natural partition layout.
  - Strict ACT table-set ordering: 16 gelus -> 1 exp -> erfs (3 loads).
  - erf args: p = -mu_x*inv (stt), two +/-5*step anchors, then +/-step
    chains on 2x-mode tensor_tensor.
  - diffs as strided tensor_tensor ops feeding fp16 DRAM output holding
    2*out; the 0.5 scale folds into the host dtype conversion.

Sharding: D split across 8 cores; partition p = b*4 + q holds
mu[b, q*1536:(q+1)*1536] of the core's D-shard.
"""

import sys

sys.path.insert(0, "/opt/trn_rl_repo")

import numpy as np

import concourse.bass as bass
import concourse.bacc as bacc
from concourse import mybir
from concourse.tile import TileContext
from concourse.bass_utils import run_bass_kernel_spmd

F32 = mybir.dt.float32
F16 = mybir.dt.float16
BF16 = mybir.dt.bfloat16
AF = mybir.ActivationFunctionType
OP = mybir.AluOpType

K = 16
SIGMA_ONE = 0.02
T_MIN = 1e-6
B, D, H = 32, 49152, 16
NCORES = 8
DS = D // NCORES          # 6144 columns per core
Q = 4                     # partitions per batch row
F = DS // Q               # 1536 free elements per partition
CHUNKS = [(0, 512), (512, 512), (1024, 256), (1280, 256)]   # consumption chunks (start, width)
LN_SQRT2 = 0.34657359027997264
NC_CONST = 32
# cn columns: 0..15 gelu biases (col 8*i+u), 16..23 gelu scales (col 16+u),
# 24 alpha, 25 negbeta, 26 lnA2, 27 lnm, 28 nb20, 29 -ln(sqrt 2)


def _build():
    nc = bacc.Bacc(None, target_bir_lowering=False)
    mu_p = nc.declare_dram_parameter("mu", [B, DS], F32, isOutput=False)
    cn_p = nc.declare_dram_parameter("cn", [128, NC_CONST], F32, isOutput=False)
    w2s_p = nc.declare_dram_parameter("w2s", [128, 16 * 64], BF16, isOutput=False)
    out_p = nc.declare_dram_parameter("out", [128, K * F], F16, isOutput=True)

    mu_v = mu_p.rearrange("b (q f) -> (b q) f", q=Q)

    with TileContext(nc) as tc:
        with (
            tc.tile_pool(name="const", bufs=1) as constp,
            tc.tile_pool(name="main", bufs=1) as mainp,
            tc.tile_pool(name="zs", bufs=1) as zspool,
            tc.tile_pool(name="hp", bufs=4) as hpool,
            tc.tile_pool(name="ap", bufs=1) as apool,
            tc.tile_pool(name="op", bufs=1) as opool,
            tc.tile_pool(name="ep", bufs=1, space="PSUM") as epool,
            tc.tile_pool(name="sp", bufs=1, space="PSUM") as spool,
        ):
            cn = constp.tile([128, NC_CONST], F32)
            w2s = constp.tile([128, 16 * 64], BF16)
            mu = mainp.tile([128, F], F32)
            zs_t = [zspool.tile([128, F], F32, name=f"zs{i}") for i in range(2)]
            # Split DMA issuing across sync (HWDGE) and gpsimd (SWDGE) so the
            # first gelu's inputs (cn + zs0) land as early as possible.
            zv0 = zs_t[0].rearrange("(g l) n -> g l n", l=2)
            zv1 = zs_t[1].rearrange("(g l) n -> g l n", l=2)
            nc.sync.dma_start(out=cn[:, :], in_=cn_p[:, :])
            nc.sync.dma_start(out=zv0[:, 0, :], in_=mu_v[0:64, :])
            nc.sync.dma_start(out=zv0[:, 1, :], in_=mu_v[0:64, :])
            nc.sync.dma_start(out=w2s[:, :], in_=w2s_p[:, :])
            nc.sync.dma_start(out=zv1[:, 0, :], in_=mu_v[64:128, :])
            nc.sync.dma_start(out=zv1[:, 1, :], in_=mu_v[64:128, :])
            nc.sync.dma_start(out=mu[:, :], in_=mu_v)

            # Warm the ACT engine on the const DMA semaphore while loading
            # the gelu table set.
            warm = constp.tile([128, 1], F16)
            nc.scalar.activation(out=warm, in_=cn[:, 0:1], func=AF.Gelu_apprx_tanh)

            mxn = mainp.tile([128, F], F16)
            v = mainp.tile([128, F], F16)
            inv = mainp.tile([128, F], F16)

            # -mu_x part 1 needs only mu -- run while ACT does the gelus
            # (cols 24/28 hold -alpha and +beta*b2[0] on the host side)
            nc.vector.tensor_scalar(
                out=mxn[:, :], in0=mu[:, :], scalar1=cn[:, 24:25],
                scalar2=cn[:, 28:29], op0=OP.mult, op1=OP.add)

            # ---- production: gelu(scale*zs + bias) -> MM2 contract (eps/sig)
            eps = epool.tile([128, F], F32)
            sig = spool.tile([128, F], F32)
            pend = None
            for i in range(2):
                for u in range(8):
                    h = hpool.tile([128, F], BF16)
                    nc.scalar.activation(
                        out=h, in_=zs_t[i], func=AF.Gelu_apprx_tanh,
                        scale=cn[:, 16 + u : 16 + u + 1],
                        bias=cn[:, 8 * i + u : 8 * i + u + 1],
                    )
                    if pend is not None:
                        _emit_mm2(nc, w2s, eps, sig, *pend)
                    pend = (h, i, u)
            _emit_mm2(nc, w2s, eps, sig, *pend)

            # ---- eps/sig consumption (DVE); v-C0 first -- it gates the
            # exp chain; later v chunks and exps overlap args-C0.
            sl0 = slice(CHUNKS[0][0], CHUNKS[0][0] + CHUNKS[0][1])
            nc.vector.tensor_scalar(
                out=v[:, sl0], in0=sig[:, sl0], scalar1=cn[:, 26:27],
                scalar2=cn[:, 27:28], op0=OP.add, op1=OP.max)
            nc.vector.scalar_tensor_tensor(
                out=mxn[:, :], in0=eps, scalar=cn[:, 25:26], in1=mxn[:, :],
                op0=OP.mult, op1=OP.add)
            for c0_, cw_ in CHUNKS[1:]:
                slc = slice(c0_, c0_ + cw_)
                nc.vector.tensor_scalar(
                    out=v[:, slc], in0=sig[:, slc], scalar1=cn[:, 26:27],
                    scalar2=cn[:, 27:28], op0=OP.add, op1=OP.max)

            # ---- inv = exp(-v - ln sqrt(2))  (one table switch, 3 chunks)
            for c0_, cw_ in CHUNKS:
                slc = slice(c0_, c0_ + cw_)
                nc.scalar.activation(
                    out=inv[:, slc], in_=v[:, slc], func=AF.Exp, scale=-1.0,
                    bias=cn[:, 29:30])

            # ---- consumption: args (DVE) / erf (ACT) / diffs+stores (DVE)
            # DVE order: argsC0, argsC1, diffsC0, argsC2, diffsC1, diffsC2
            # ACT order: erfC0, erfC1, erfC2 (table already on erf set)
            a_t = {}

            def emit_args(c):
                c0, cw = CHUNKS[c]
                sl = slice(c0, c0 + cw)
                a = apool.tile([128, 15, cw], F16, name=f"a{c}", tag=f"a{c}")
                a_t[c] = a
                s1 = apool.tile([128, cw], F16, name=f"s1_{c}", tag=f"s1_{c}")
                s5 = apool.tile([128, cw], F16, name=f"s5_{c}", tag=f"s5_{c}")
                pt = apool.tile([128, cw], F16, name=f"pt_{c}", tag=f"pt_{c}")

                def step(src_, dst):
                    in0 = pt if src_ == 7 else a[:, src_, :]
                    nc.vector.tensor_tensor(
                        out=a[:, dst, :], in0=in0, in1=s1,
                        op=OP.add if dst > src_ else OP.subtract)

                # erf ops are interleaved with the arg chains, ordered by
                # argument readiness so ACT starts earlier. erf runs in
                # place, so the shared anchor p lives in its own tile (pt);
                # anything read after an erf of its slice must come from pt.
                nc.vector.tensor_scalar_mul(out=s1, in0=inv[:, sl], scalar1=0.125)
                nc.vector.tensor_tensor(
                    out=pt, in0=mxn[:, sl], in1=inv[:, sl], op=OP.mult)
                nc.vector.tensor_copy(a[:, 7, :], pt)
                step(7, 6)
                step(6, 5)
                nc.scalar.activation(out=a[:, 5:8, :], in_=a[:, 5:8, :], func=AF.Erf)
                step(7, 8)
                step(8, 9)
                nc.scalar.activation(out=a[:, 8:10, :], in_=a[:, 8:10, :], func=AF.Erf)
                nc.vector.tensor_scalar_mul(out=s5, in0=inv[:, sl], scalar1=0.625)
                nc.vector.tensor_tensor(
                    out=a[:, 2, :], in0=pt, in1=s5, op=OP.subtract)
                step(2, 1)
                step(1, 0)
                step(2, 3)
                step(3, 4)
                nc.scalar.activation(out=a[:, 0:5, :], in_=a[:, 0:5, :], func=AF.Erf)
                nc.vector.tensor_tensor(
                    out=a[:, 12, :], in0=pt, in1=s5, op=OP.add)
                step(12, 11)
                step(11, 10)
                step(12, 13)
                step(13, 14)
                nc.scalar.activation(out=a[:, 10:15, :], in_=a[:, 10:15, :], func=AF.Erf)

            def emit_diffs(c):
                c0, cw = CHUNKS[c]
                eng = nc.sync
                a = a_t[c]
                o = opool.tile([128, K, cw], F16, name=f"o{c}", tag=f"o{c}")
                nc.vector.tensor_scalar_add(out=o[:, 0, :], in0=a[:, 0, :], scalar1=1.0)
                nc.vector.tensor_tensor(
                    out=o[:, 1:4, :], in0=a[:, 1:4, :], in1=a[:, 0:3, :],
                    op=OP.subtract)
                nc.vector.tensor_tensor(
                    out=o[:, 4:8, :], in0=a[:, 4:8, :], in1=a[:, 3:7, :],
                    op=OP.subtract)
                eng.dma_start(
                    out=out_p[:, K * c0 : K * c0 + 8 * cw],
                    in_=o[:, 0:8, :].rearrange("p k n -> p (k n)"))
                nc.vector.tensor_tensor(
                    out=o[:, 8:12, :], in0=a[:, 8:12, :], in1=a[:, 7:11, :],
                    op=OP.subtract)
                nc.vector.tensor_scalar(
                    out=o[:, 15, :], in0=a[:, 14, :], scalar1=-1.0, scalar2=1.0,
                    op0=OP.mult, op1=OP.add)
                nc.vector.tensor_tensor(
                    out=o[:, 12:15, :], in0=a[:, 12:15, :], in1=a[:, 11:14, :],
                    op=OP.subtract)
                eng.dma_start(
                    out=out_p[:, K * c0 + 8 * cw : K * c0 + 16 * cw],
                    in_=o[:, 8:16, :].rearrange("p k n -> p (k n)"))

            emit_args(0)
            emit_args(1)
            emit_diffs(0)
            emit_args(2)
            emit_diffs(1)
            emit_args(3)
            emit_diffs(2)
            emit_diffs(3)

    return nc


def _emit_mm2(nc, w2s, eps, sig, h, i, u):
    st = u == 0
    sp = u == 7
    for col, dst in ((1, sig), (0, eps)):
        lhs2 = w2s[:, (2 * u + col) * 64 : (2 * u + col) * 64 + 64]
        for c in range(3):
            nc.tensor.matmul(
                dst[64 * i : 64 * i + 64, 512 * c : 512 * (c + 1)], lhs2,
                h[:, 512 * c : 512 * (c + 1)], start=st, stop=sp)


def _host_consts(t, W1, b1, W2, b2):
    t64 = np.asarray(t, np.float64).reshape(B)
    cond = t64 < T_MIN
    gamma = 1.0 - SIGMA_ONE ** (2.0 * t64)
    alpha = np.where(cond, 0.0, 1.0 / np.where(gamma == 0, 1.0, gamma))
    beta = np.sqrt(np.maximum(1.0 - gamma, 0.0) / np.where(gamma == 0, 1.0, gamma))
    negbeta = np.where(cond, 0.0, -beta)
    lnA2 = np.where(cond, -1e4, np.log(np.maximum(beta, 1e-300)) + float(b2[1]))
    lnm = np.where(cond, 0.0, np.log(SIGMA_ONE))
    nb20 = np.where(cond, 0.0, -beta * float(b2[0]))

    cn = np.zeros((128, NC_CONST), np.float32)
    # gelu biases/scales at expanded layout: partition p = 2*g + l handles
    # source row s = 64*i + g (batch b = s//4) and unit j = 2*u + l.
    cvals = (t64[:, None] * np.asarray(W1, np.float64)[1, :][None, :]
             + np.asarray(b1, np.float64)[None, :])          # [B, H]
    for u in range(8):
        for g in range(64):
            for l in range(2):
                p = 2 * g + l
                cn[p, 16 + u] = W1[0, 2 * u + l]
                for i in range(2):
                    cn[p, 8 * i + u] = cvals[16 * i + g // 4, 2 * u + l]
    # mxn = -mu_x accumulates as (-alpha)*mu + beta*b2[0] + beta*eps
    for p in range(128):
        bb = p // Q
        cn[p, 24] = -alpha[bb]
        cn[p, 25] = -negbeta[bb]
        cn[p, 26] = lnA2[bb]
        cn[p, 27] = lnm[bb]
        cn[p, 28] = -nb20[bb]
        cn[p, 29] = -LN_SQRT2

    # MM2 stationary: w2s[2*g+l, (2u+col)*64 + m] = (g == m) * W2[2u+l, col]
    w2s = np.zeros((128, 16 * 64), np.float32)
    for u in range(8):
        for col in range(2):
            s0 = (2 * u + col) * 64
            for m in range(64):
                for l in range(2):
                    w2s[2 * m + l, s0 + m] = W2[2 * u + l, col]
    import ml_dtypes
    w2s = w2s.astype(ml_dtypes.bfloat16)

    return cn, w2s


def _run(inputs, trace=False):
    mu = np.ascontiguousarray(np.asarray(inputs["mu"], np.float32))
    t = np.asarray(inputs["t"], np.float32)
    W1 = np.asarray(inputs["W1"], np.float32)
    b1 = np.asarray(inputs["b1"], np.float32)
    W2 = np.asarray(inputs["W2"], np.float32)
    b2 = np.asarray(inputs["b2"], np.float32)

    nc = _build()
    nc.finalize()
    cn, w2s = _host_consts(t, W1, b1, W2, b2)

    in_maps = []
    for c in range(NCORES):
        shard = np.ascontiguousarray(mu[:, c * DS : (c + 1) * DS])
        in_maps.append({"mu": shard, "cn": cn, "w2s": w2s})

    res = run_bass_kernel_spmd(nc, in_maps, list(range(NCORES)), trace=trace)
    shards = []
    for c in range(NCORES):
        s = np.asarray(res.results[c]["out"])          # [128, K*F] f16 (2*out)
        blocks = []
        for c0, cw in CHUNKS:
            blocks.append(s[:, K * c0 : K * (c0 + cw)].reshape(128, K, cw))
        s = np.concatenate(blocks, axis=2)             # [128, K, F]
        s = s.reshape(B, Q, K, F).transpose(0, 1, 3, 2).reshape(B, DS, K)
        shards.append(s)
    out = np.concatenate(shards, axis=1).astype(np.float32)
    out *= np.float32(0.5)
    return np.ascontiguousarray(out), res


def kernel(**inputs) -> np.ndarray:
    out, _ = _run(inputs, trace=False)
    return out


if __name__ == "__main__":
    rng = np.random.default_rng(0)
    demo = {
        "mu": rng.standard_normal((B, D), dtype=np.float32),
        "t": rng.random((B, 1), dtype=np.float32),
        "W1": rng.standard_normal((2, H), dtype=np.float32) * 0.5,
        "b1": rng.standard_normal((H,), dtype=np.float32) * 0.1,
        "W2": rng.standard_normal((H, 2), dtype=np.float32) * 0.1,
        "b2": rng.standard_normal((2,), dtype=np.float32) * 0.1,
    }
    out = kernel(**demo)
    print("kernel output", out.shape, out.dtype, out[0, 0])


# revision 4
# speedup vs baseline: 1.1743x; 1.0007x over previous
"""Trainium2 Bass kernel for nn_BayesianFlowNetworkDiscretised (v3).

Per (b, d) position:
    MLP: h_j = gelu_tanh(W1[0,j]*mu + t*W1[1,j] + b1[j]);  (mu_eps, ln_sig) = h@W2 + b2
    mu_x = mu/gamma - var_scale*mu_eps
    sigma = max(var_scale*exp(ln_sig), 0.02)
    out_k = Phi((e_{k+1}-mu_x)/sigma) - Phi((e_k-mu_x)/sigma),  e_i = i/8 - 1

Structure:
  - mu is partition-replicated 2x by DMA straight from DRAM (stride-2
    partition APs) into two expanded blocks; the per-unit W1 scale and the
    per-(batch,unit) bias ride free in the gelu ACTIVATE (per-partition
    scale/bias APs), so layer 1 costs nothing beyond the intrinsic gelus.
  - Layer 2 runs on the TensorEngine: per (block i, unit-pair u), bf16
    matmuls with W2-scatter stationaries accumulate eps/sig into PSUM in
    the natural partition layout.
  - Strict ACT table-set ordering: 16 gelus -> 1 exp -> erfs (3 loads).
  - erf args: p = -mu_x*inv (stt), two +/-5*step anchors, then +/-step
    chains on 2x-mode tensor_tensor.
  - diffs as strided tensor_tensor ops feeding fp16 DRAM output holding
    2*out; the 0.5 scale folds into the host dtype conversion.

Sharding: D split across 8 cores; partition p = b*4 + q holds
mu[b, q*1536:(q+1)*1536] of the core's D-shard.
"""

import sys

sys.path.insert(0, "/opt/trn_rl_repo")

import numpy as np

import concourse.bass as bass
import concourse.bacc as bacc
from concourse import mybir
from concourse.tile import TileContext
from concourse.bass_utils import run_bass_kernel_spmd

F32 = mybir.dt.float32
F16 = mybir.dt.float16
BF16 = mybir.dt.bfloat16
AF = mybir.ActivationFunctionType
OP = mybir.AluOpType

K = 16
SIGMA_ONE = 0.02
T_MIN = 1e-6
B, D, H = 32, 49152, 16
NCORES = 8
DS = D // NCORES          # 6144 columns per core
Q = 4                     # partitions per batch row
F = DS // Q               # 1536 free elements per partition
CHUNKS = [(0, 512), (512, 512), (1024, 384), (1408, 128)]   # consumption chunks (start, width)
LN_SQRT2 = 0.34657359027997264
NC_CONST = 32
# cn columns: 0..15 gelu biases (col 8*i+u), 16..23 gelu scales (col 16+u),
# 24 alpha, 25 negbeta, 26 lnA2, 27 lnm, 28 nb20, 29 -ln(sqrt 2)


def _build():
    nc = bacc.Bacc(None, target_bir_lowering=False)
    mu_p = nc.declare_dram_parameter("mu", [B, DS], F32, isOutput=False)
    cn_p = nc.declare_dram_parameter("cn", [128, NC_CONST], F32, isOutput=False)
    w2s_p = nc.declare_dram_parameter("w2s", [128, 16 * 64], BF16, isOutput=False)
    out_p = nc.declare_dram_parameter("out", [128, K * F], F16, isOutput=True)

    mu_v = mu_p.rearrange("b (q f) -> (b q) f", q=Q)

    with TileContext(nc) as tc:
        with (
            tc.tile_pool(name="const", bufs=1) as constp,
            tc.tile_pool(name="main", bufs=1) as mainp,
            tc.tile_pool(name="zs", bufs=1) as zspool,
            tc.tile_pool(name="hp", bufs=4) as hpool,
            tc.tile_pool(name="ap", bufs=1) as apool,
            tc.tile_pool(name="op", bufs=1) as opool,
            tc.tile_pool(name="ep", bufs=1, space="PSUM") as epool,
            tc.tile_pool(name="sp", bufs=1, space="PSUM") as spool,
        ):
            cn = constp.tile([128, NC_CONST], F32)
            w2s = constp.tile([128, 16 * 64], BF16)
            mu = mainp.tile([128, F], F32)
            zs_t = [zspool.tile([128, F], F32, name=f"zs{i}") for i in range(2)]
            # Split DMA issuing across sync (HWDGE) and gpsimd (SWDGE) so the
            # first gelu's inputs (cn + zs0) land as early as possible.
            zv0 = zs_t[0].rearrange("(g l) n -> g l n", l=2)
            zv1 = zs_t[1].rearrange("(g l) n -> g l n", l=2)
            nc.sync.dma_start(out=cn[:, :], in_=cn_p[:, :])
            nc.sync.dma_start(out=zv0[:, 0, :], in_=mu_v[0:64, :])
            nc.sync.dma_start(out=zv0[:, 1, :], in_=mu_v[0:64, :])
            nc.sync.dma_start(out=w2s[:, :], in_=w2s_p[:, :])
            nc.sync.dma_start(out=zv1[:, 0, :], in_=mu_v[64:128, :])
            nc.sync.dma_start(out=zv1[:, 1, :], in_=mu_v[64:128, :])
            nc.sync.dma_start(out=mu[:, :], in_=mu_v)

            # Warm the ACT engine on the const DMA semaphore while loading
            # the gelu table set.
            warm = constp.tile([128, 1], F16)
            nc.scalar.activation(out=warm, in_=cn[:, 0:1], func=AF.Gelu_apprx_tanh)

            mxn = mainp.tile([128, F], F16)
            v = mainp.tile([128, F], F16)
            inv = mainp.tile([128, F], F16)

            # -mu_x part 1 needs only mu -- run while ACT does the gelus
            # (cols 24/28 hold -alpha and +beta*b2[0] on the host side)
            nc.vector.tensor_scalar(
                out=mxn[:, :], in0=mu[:, :], scalar1=cn[:, 24:25],
                scalar2=cn[:, 28:29], op0=OP.mult, op1=OP.add)

            # ---- production: gelu(scale*zs + bias) -> MM2 contract (eps/sig)
            eps = epool.tile([128, F], F32)
            sig = spool.tile([128, F], F32)
            pend = None
            for i in range(2):
                for u in range(8):
                    h = hpool.tile([128, F], BF16)
                    nc.scalar.activation(
                        out=h, in_=zs_t[i], func=AF.Gelu_apprx_tanh,
                        scale=cn[:, 16 + u : 16 + u + 1],
                        bias=cn[:, 8 * i + u : 8 * i + u + 1],
                    )
                    if pend is not None:
                        _emit_mm2(nc, w2s, eps, sig, *pend)
                    pend = (h, i, u)
            _emit_mm2(nc, w2s, eps, sig, *pend)

            # ---- eps/sig consumption (DVE); v-C0 first -- it gates the
            # exp chain; later v chunks and exps overlap args-C0.
            sl0 = slice(CHUNKS[0][0], CHUNKS[0][0] + CHUNKS[0][1])
            nc.vector.tensor_scalar(
                out=v[:, sl0], in0=sig[:, sl0], scalar1=cn[:, 26:27],
                scalar2=cn[:, 27:28], op0=OP.add, op1=OP.max)
            nc.vector.scalar_tensor_tensor(
                out=mxn[:, :], in0=eps, scalar=cn[:, 25:26], in1=mxn[:, :],
                op0=OP.mult, op1=OP.add)
            for c0_, cw_ in CHUNKS[1:]:
                slc = slice(c0_, c0_ + cw_)
                nc.vector.tensor_scalar(
                    out=v[:, slc], in0=sig[:, slc], scalar1=cn[:, 26:27],
                    scalar2=cn[:, 27:28], op0=OP.add, op1=OP.max)

            # ---- inv = exp(-v - ln sqrt(2))  (one table switch, 3 chunks)
            for c0_, cw_ in CHUNKS:
                slc = slice(c0_, c0_ + cw_)
                nc.scalar.activation(
                    out=inv[:, slc], in_=v[:, slc], func=AF.Exp, scale=-1.0,
                    bias=cn[:, 29:30])

            # ---- consumption: args (DVE) / erf (ACT) / diffs+stores (DVE)
            # DVE order: argsC0, argsC1, diffsC0, argsC2, diffsC1, diffsC2
            # ACT order: erfC0, erfC1, erfC2 (table already on erf set)
            a_t = {}

            def emit_args(c):
                c0, cw = CHUNKS[c]
                sl = slice(c0, c0 + cw)
                a = apool.tile([128, 15, cw], F16, name=f"a{c}", tag=f"a{c}")
                a_t[c] = a
                s1 = apool.tile([128, cw], F16, name=f"s1_{c}", tag=f"s1_{c}")
                s5 = apool.tile([128, cw], F16, name=f"s5_{c}", tag=f"s5_{c}")
                pt = apool.tile([128, cw], F16, name=f"pt_{c}", tag=f"pt_{c}")

                def step(src_, dst):
                    in0 = pt if src_ == 7 else a[:, src_, :]
                    nc.vector.tensor_tensor(
                        out=a[:, dst, :], in0=in0, in1=s1,
                        op=OP.add if dst > src_ else OP.subtract)

                # erf ops are interleaved with the arg chains, ordered by
                # argument readiness so ACT starts earlier. erf runs in
                # place, so the shared anchor p lives in its own tile (pt);
                # anything read after an erf of its slice must come from pt.
                nc.vector.tensor_scalar_mul(out=s1, in0=inv[:, sl], scalar1=0.125)
                nc.vector.tensor_tensor(
                    out=pt, in0=mxn[:, sl], in1=inv[:, sl], op=OP.mult)
                nc.vector.tensor_copy(a[:, 7, :], pt)
                step(7, 6)
                step(6, 5)
                nc.scalar.activation(out=a[:, 5:8, :], in_=a[:, 5:8, :], func=AF.Erf)
                step(7, 8)
                step(8, 9)
                nc.scalar.activation(out=a[:, 8:10, :], in_=a[:, 8:10, :], func=AF.Erf)
                nc.vector.tensor_scalar_mul(out=s5, in0=inv[:, sl], scalar1=0.625)
                nc.vector.tensor_tensor(
                    out=a[:, 2, :], in0=pt, in1=s5, op=OP.subtract)
                step(2, 1)
                step(1, 0)
                step(2, 3)
                step(3, 4)
                nc.scalar.activation(out=a[:, 0:5, :], in_=a[:, 0:5, :], func=AF.Erf)
                nc.vector.tensor_tensor(
                    out=a[:, 12, :], in0=pt, in1=s5, op=OP.add)
                step(12, 11)
                step(11, 10)
                step(12, 13)
                step(13, 14)
                nc.scalar.activation(out=a[:, 10:15, :], in_=a[:, 10:15, :], func=AF.Erf)

            def emit_diffs(c):
                c0, cw = CHUNKS[c]
                eng = nc.sync
                a = a_t[c]
                o = opool.tile([128, K, cw], F16, name=f"o{c}", tag=f"o{c}")
                nc.vector.tensor_scalar_add(out=o[:, 0, :], in0=a[:, 0, :], scalar1=1.0)
                nc.vector.tensor_tensor(
                    out=o[:, 1:4, :], in0=a[:, 1:4, :], in1=a[:, 0:3, :],
                    op=OP.subtract)
                nc.vector.tensor_tensor(
                    out=o[:, 4:8, :], in0=a[:, 4:8, :], in1=a[:, 3:7, :],
                    op=OP.subtract)
                eng.dma_start(
                    out=out_p[:, K * c0 : K * c0 + 8 * cw],
                    in_=o[:, 0:8, :].rearrange("p k n -> p (k n)"))
                nc.vector.tensor_tensor(
                    out=o[:, 8:12, :], in0=a[:, 8:12, :], in1=a[:, 7:11, :],
                    op=OP.subtract)
                nc.vector.tensor_scalar(
                    out=o[:, 15, :], in0=a[:, 14, :], scalar1=-1.0, scalar2=1.0,
                    op0=OP.mult, op1=OP.add)
                nc.vector.tensor_tensor(
                    out=o[:, 12:15, :], in0=a[:, 12:15, :], in1=a[:, 11:14, :],
                    op=OP.subtract)
                eng.dma_start(
                    out=out_p[:, K * c0 + 8 * cw : K * c0 + 16 * cw],
                    in_=o[:, 8:16, :].rearrange("p k n -> p (k n)"))

            emit_args(0)
            emit_args(1)
            emit_diffs(0)
            emit_args(2)
            emit_diffs(1)
            emit_args(3)
            emit_diffs(2)
            emit_diffs(3)

    return nc


def _emit_mm2(nc, w2s, eps, sig, h, i, u):
    st = u == 0
    sp = u == 7
    for col, dst in ((1, sig), (0, eps)):
        lhs2 = w2s[:, (2 * u + col) * 64 : (2 * u + col) * 64 + 64]
        for c in range(3):
            nc.tensor.matmul(
                dst[64 * i : 64 * i + 64, 512 * c : 512 * (c + 1)], lhs2,
                h[:, 512 * c : 512 * (c + 1)], start=st, stop=sp)


def _host_consts(t, W1, b1, W2, b2):
    t64 = np.asarray(t, np.float64).reshape(B)
    cond = t64 < T_MIN
    gamma = 1.0 - SIGMA_ONE ** (2.0 * t64)
    alpha = np.where(cond, 0.0, 1.0 / np.where(gamma == 0, 1.0, gamma))
    beta = np.sqrt(np.maximum(1.0 - gamma, 0.0) / np.where(gamma == 0, 1.0, gamma))
    negbeta = np.where(cond, 0.0, -beta)
    lnA2 = np.where(cond, -1e4, np.log(np.maximum(beta, 1e-300)) + float(b2[1]))
    lnm = np.where(cond, 0.0, np.log(SIGMA_ONE))
    nb20 = np.where(cond, 0.0, -beta * float(b2[0]))

    cn = np.zeros((128, NC_CONST), np.float32)
    # gelu biases/scales at expanded layout: partition p = 2*g + l handles
    # source row s = 64*i + g (batch b = s//4) and unit j = 2*u + l.
    cvals = (t64[:, None] * np.asarray(W1, np.float64)[1, :][None, :]
             + np.asarray(b1, np.float64)[None, :])          # [B, H]
    for u in range(8):
        for g in range(64):
            for l in range(2):
                p = 2 * g + l
                cn[p, 16 + u] = W1[0, 2 * u + l]
                for i in range(2):
                    cn[p, 8 * i + u] = cvals[16 * i + g // 4, 2 * u + l]
    # mxn = -mu_x accumulates as (-alpha)*mu + beta*b2[0] + beta*eps
    for p in range(128):
        bb = p // Q
        cn[p, 24] = -alpha[bb]
        cn[p, 25] = -negbeta[bb]
        cn[p, 26] = lnA2[bb]
        cn[p, 27] = lnm[bb]
        cn[p, 28] = -nb20[bb]
        cn[p, 29] = -LN_SQRT2

    # MM2 stationary: w2s[2*g+l, (2u+col)*64 + m] = (g == m) * W2[2u+l, col]
    w2s = np.zeros((128, 16 * 64), np.float32)
    for u in range(8):
        for col in range(2):
            s0 = (2 * u + col) * 64
            for m in range(64):
                for l in range(2):
                    w2s[2 * m + l, s0 + m] = W2[2 * u + l, col]
    import ml_dtypes
    w2s = w2s.astype(ml_dtypes.bfloat16)

    return cn, w2s


def _run(inputs, trace=False):
    mu = np.ascontiguousarray(np.asarray(inputs["mu"], np.float32))
    t = np.asarray(inputs["t"], np.float32)
    W1 = np.asarray(inputs["W1"], np.float32)
    b1 = np.asarray(inputs["b1"], np.float32)
    W2 = np.asarray(inputs["W2"], np.float32)
    b2 = np.asarray(inputs["b2"], np.float32)

    nc = _build()
    nc.finalize()
    cn, w2s = _host_consts(t, W1, b1, W2, b2)

    in_maps = []
    for c in range(NCORES):
        shard = np.ascontiguousarray(mu[:, c * DS : (c + 1) * DS])
        in_maps.append({"mu": shard, "cn": cn, "w2s": w2s})

    res = run_bass_kernel_spmd(nc, in_maps, list(range(NCORES)), trace=trace)
    shards = []
    for c in range(NCORES):
        s = np.asarray(res.results[c]["out"])          # [128, K*F] f16 (2*out)
        blocks = []
        for c0, cw in CHUNKS:
            blocks.append(s[:, K * c0 : K * (c0 + cw)].reshape(128, K, cw))
        s = np.concatenate(blocks, axis=2)             # [128, K, F]
        s = s.reshape(B, Q, K, F).transpose(0, 1, 3, 2).reshape(B, DS, K)
        shards.append(s)
    out = np.concatenate(shards, axis=1).astype(np.float32)
    out *= np.float32(0.5)
    return np.ascontiguousarray(out), res


def kernel(**inputs) -> np.ndarray:
    out, _ = _run(inputs, trace=False)
    return out


if __name__ == "__main__":
    rng = np.random.default_rng(0)
    demo = {
        "mu": rng.standard_normal((B, D), dtype=np.float32),
        "t": rng.random((B, 1), dtype=np.float32),
        "W1": rng.standard_normal((2, H), dtype=np.float32) * 0.5,
        "b1": rng.standard_normal((H,), dtype=np.float32) * 0.1,
        "W2": rng.standard_normal((H, 2), dtype=np.float32) * 0.1,
        "b2": rng.standard_normal((2,), dtype=np.float32) * 0.1,
    }
    out = kernel(**demo)
    print("kernel output", out.shape, out.dtype, out[0, 0])
